# Initial kernel scaffold
#
"""Trainium2 Bass kernel for nn_LocalSolverCore (sparse local-window attention solver).

Sharding: 8 cores = 2 batches x 4 sequence-chunks of 512 tokens.
Per transformer block: AllGather halo exchange (128 tokens each side) within
each batch group of 4 cores; banded attention computed in transposed score
layout [k_part, q_free]; probs/masks/V in bf16; matmuls in float32r.
LN gains/biases are folded into the following weight matrices host-side.
"""

import os
import numpy as np
import ml_dtypes

import concourse.bass as bass
import concourse.mybir as mybir
import concourse.tile as tile
from concourse import bacc
from concourse.bass_utils import run_bass_kernel_spmd

BF16 = mybir.dt.bfloat16
F32 = mybir.dt.float32
F32R = mybir.dt.float32r
I32 = mybir.dt.int32
NPBF16 = ml_dtypes.bfloat16
AF = mybir.ActivationFunctionType
ALU = mybir.AluOpType

B, T, D_IN, D = 2, 2048, 4096, 512
H, DH, W_WIN, NM = 8, 64, 128, 16
K_OUTER, K_INNER = 3, 4
NH_X, DH_X = 4, 128
EPS = 1e-5
C = 512
EXT = C + 2 * W_WIN          # 768
NT_OWN, NT_EXT = 4, 6
NDC = 4                      # D/128
N_DIN = 32                   # D_IN/128
NCHUNK = 4

# k-tile j (ext rows [128j,128j+128)) -> q window [qlo, qhi)
QWIN = [(0, 128), (0, 256), (0, 384), (128, 512), (256, 512), (384, 512)]
# PV first-writer splits per j: (qa, qb, start)
PV_SPLITS = [
    [(0, 128, True)],
    [(0, 128, False), (128, 256, True)],
    [(0, 256, False), (256, 384, True)],
    [(128, 384, False), (384, 512, True)],
    [(256, 512, False)],
    [(384, 512, False)],
]
GROUPS = [[0, 1, 2, 3], [4, 5, 6, 7]]

_CACHE = {}


def _build_program():
    nc = bacc.Bacc(None, target_bir_lowering=False)

    def inp(name, shape, dt=F32):
        return nc.dram_tensor(name, list(shape), dt, kind="ExternalInput")

    promptT_d = inp("promptT", [N_DIN, 128, C], F32R)
    proj_in_d = inp("proj_in", [N_DIN, 128, D], F32R)
    wq_d = inp("wq", [128, NDC, D], F32R)
    wk_d = inp("wk", [128, NDC, D], F32R)
    wv_d = inp("wv", [128, NDC, D], F32R)
    wo_d = inp("wo", [64, H, D], BF16)
    wg_d = inp("wg", [128, NDC, 2 * D], F32R)
    wd_d = inp("wd", [128, 8, D], F32R)
    hk_d = inp("hk_w", [128, NDC, D], F32R)
    hv_d = inp("hv_w", [128, NDC, D], F32R)
    hq_d = inp("hq_w", [128, NDC, D], F32R)
    how_d = inp("ho_w", [128, NDC, D], F32R)
    po_d = inp("proj_out", [NDC, 8, 128, D], F32R)
    bqk_d = inp("bqk", [128, NDC, 2])
    bg_d = inp("bg_t", [128, 8])
    bhq_d = inp("bhq_t", [128, NDC])
    bv_d = inp("bv_bc", [128, D])
    mask_d = inp("masks", [128, 6, C], BF16)
    hidx_d = inp("halo_idx", [128, 2], I32)
    id_d = inp("identity", [128, 128])
    ones_d = inp("onesc", [128, 128], F32R)
    zh0_d = inp("h_init_bc", [NM, D])
    ong_d = inp("on_g_bc", [NM, D_IN], BF16)
    onb_d = inp("on_b_bc", [NM, D_IN], BF16)

    out_y = nc.dram_tensor("out_y", [NM, D_IN], F32, kind="ExternalOutput")
    dbg_d = nc.dram_tensor("dbg", [8, 128, D], F32, kind="ExternalOutput")

    ag_in = nc.dram_tensor("ag_in", [256, D], F32)
    ag_out = nc.dram_tensor("ag_out", [1024, D], F32)
    ar_in = nc.dram_tensor("ar_in", [NH_X * DH_X + NH_X, NM], F32)
    ar_out = nc.dram_tensor("ar_out", [NH_X * DH_X + NH_X, NM], F32)
    y_scr = nc.dram_tensor("y_scr", [NM, D_IN], F32)

    import contextlib
    with nc.allow_low_precision(reason="bf16 probs/f32r matmul operands are intentional"), \
            tile.TileContext(nc) as tc, contextlib.ExitStack() as ctx:
        singles = ctx.enter_context(tc.tile_pool(name="singles", bufs=1))
        psA = ctx.enter_context(tc.tile_pool(name="psA", bufs=4, space="PSUM"))
        psS = ctx.enter_context(tc.tile_pool(name="psS", bufs=2, space="PSUM"))
        psO = ctx.enter_context(tc.tile_pool(name="psO", bufs=1, space="PSUM"))
        work = ctx.enter_context(tc.tile_pool(name="work", bufs=1))
        htok = ctx.enter_context(tc.tile_pool(name="htok", bufs=2))
        small = ctx.enter_context(tc.tile_pool(name="small", bufs=2))
        stream = ctx.enter_context(tc.tile_pool(name="stream", bufs=2))

        def load(name, ap, shape, dt=F32):
            t = singles.tile(list(shape), dt, tag=name)
            nc.sync.dma_start(out=t[:], in_=ap)
            return t

        wq_sb = load("wq", wq_d[:], [128, NDC, D], F32R)
        wk_sb = load("wk", wk_d[:], [128, NDC, D], F32R)
        wv_sb = load("wv", wv_d[:], [128, NDC, D], F32R)
        wo_sb = load("wo", wo_d[:], [64, H, D], BF16)
        wg_sb = load("wg", wg_d[:], [128, NDC, 2 * D], F32R)
        wd_sb = load("wd", wd_d[:], [128, 8, D], F32R)
        bqk_sb = load("bqk", bqk_d[:], [128, NDC, 2])
        bg_sb = load("bg_t", bg_d[:], [128, 8])
        bhq_sb = load("bhq_t", bhq_d[:], [128, NDC])
        bv_sb = load("bv_bc", bv_d[:], [128, D])
        mask_sb = load("masks", mask_d[:], [128, 6, C], BF16)
        hidx_sb = load("halo_idx", hidx_d[:], [128, 2], I32)
        id_sb = load("identity", id_d[:], [128, 128])
        ones_sb = load("onesc", ones_d[:], [128, 128], F32R)
        zh_sb = load("h_init_bc", zh0_d[:], [NM, D])

        eps_sb = singles.tile([128, 1], F32, tag="eps")
        nc.vector.memset(eps_sb[:], EPS)

        e_sb = singles.tile([128, NT_OWN, D], F32, tag="e")
        x_sb = singles.tile([128, NT_OWN, D], F32, tag="x")
        hal_sb = singles.tile([128, 2, D], F32, tag="hal")
        v_sb = singles.tile([128, NT_EXT, H, DH + 1], BF16, tag="v")
        v2_sb = singles.tile([128, NT_OWN, NH_X, DH_X + 1], BF16, tag="v2")
        nc.vector.memset(v_sb[:, :, :, DH:DH + 1], 1.0)
        pT_sb = singles.tile([128, 6, C], BF16, tag="pTs")
        nc.gpsimd.memset(pT_sb[:], 0.0)
        nc.vector.memset(v2_sb[:, :, :, DH_X:DH_X + 1], 1.0)

        # ----- e = promptT.T @ proj_in (token-major); z_L = 0.1 e -----
        e_ps = [psA.tile([128, D], F32, tag="a", name=f"e_ps{i}") for i in range(NT_OWN)]
        for dc in range(N_DIN):
            pt_t = stream.tile([128, C], F32R, tag="pt")
            pi_t = stream.tile([128, D], F32R, tag="pi")
            nc.sync.dma_start(out=pt_t[:], in_=promptT_d[dc])
            nc.sync.dma_start(out=pi_t[:], in_=proj_in_d[dc])
            for tt in range(NT_OWN):
                nc.tensor.matmul(
                    e_ps[tt][:], pt_t[:, tt * 128:(tt + 1) * 128], pi_t[:],
                    start=(dc == 0), stop=(dc == N_DIN - 1))
        for tt in range(NT_OWN):
            nc.scalar.copy(out=e_sb[:, tt, :], in_=e_ps[tt][:])
            nc.scalar.mul(out=x_sb[:, tt, :], in_=e_ps[tt][:], mul=0.1)

        def layernorm_tile(src_ap, dst_ap, np_=128):
            st = small.tile([np_, 6], F32, tag="bnst")
            mv = small.tile([np_, 2], F32, tag="bnmv")
            rs = small.tile([np_, 1], F32, tag="rstd")
            nc.vector.bn_stats(out=st[:], in_=src_ap)
            nc.vector.bn_aggr(out=mv[:], in_=st[:])
            nc.scalar.activation(out=rs[:], in_=mv[:, 1:2], func=AF.Sqrt,
                                 bias=eps_sb[0:np_, :], scale=1.0)
            nc.vector.reciprocal(out=rs[:], in_=rs[:])
            nc.vector.tensor_scalar(out=dst_ap, in0=src_ap,
                                    scalar1=mv[:, 0:1], scalar2=rs[:],
                                    op0=ALU.subtract, op1=ALU.mult)

        def transpose_block(get_src, n_tt, dst_sb):
            """dst_sb[:, ds, tt*128+...] = src(tt)[:, ds*128+...].T via PE."""
            for t0 in range(0, n_tt, 4):
                tts = list(range(t0, min(t0 + 4, n_tt)))
                pss = [psA.tile([128, 512], F32, tag="a", name=f"tp{i}") for i in range(NDC)]
                for tt in tts:
                    src = get_src(tt)
                    for ds in range(NDC):
                        nc.tensor.transpose(
                            pss[ds][:, (tt - t0) * 128:(tt - t0 + 1) * 128],
                            src[:, ds * 128:(ds + 1) * 128], id_sb[:])
                w = len(tts) * 128
                for ds in range(NDC):
                    nc.scalar.copy(out=dst_sb[:, ds, t0 * 128:t0 * 128 + w],
                                   in_=pss[ds][:, :w])

        # ================= main iteration =================
        for s_outer in range(K_OUTER):
            for s_inner in range(K_INNER):
                # x_in = z_L + e  (in place; the +e is part of the block output)
                for tt in range(NT_OWN):
                    nc.vector.tensor_add(out=x_sb[:, tt, :],
                                         in0=x_sb[:, tt, :], in1=e_sb[:, tt, :])
                # halo exchange of boundary token tiles
                nc.sync.dma_start(out=ag_in[0:128, :], in_=x_sb[:, 0, :])
                nc.sync.dma_start(out=ag_in[128:256, :], in_=x_sb[:, 3, :])
                nc.gpsimd.collective_compute(
                    "AllGather", ALU.bypass, ins=[ag_in[:]], outs=[ag_out[:]],
                    replica_groups=GROUPS)
                nc.gpsimd.indirect_dma_start(
                    out=hal_sb[:, 0, :], out_offset=None, in_=ag_out[:],
                    in_offset=bass.IndirectOffsetOnAxis(ap=hidx_sb[:, 0:1],
                                                        axis=0))
                nc.gpsimd.indirect_dma_start(
                    out=hal_sb[:, 1, :], out_offset=None, in_=ag_out[:],
                    in_offset=bass.IndirectOffsetOnAxis(ap=hidx_sb[:, 1:2],
                                                        axis=0))

                if s_outer == 0 and s_inner == 0:
                    nc.sync.dma_start(out=dbg_d[0], in_=e_sb[:, 0, :])
                    nc.sync.dma_start(out=dbg_d[1], in_=x_sb[:, 0, :])
                    nc.sync.dma_start(out=dbg_d[2], in_=hal_sb[:, 0, :])
                    nc.sync.dma_start(out=dbg_d[3], in_=hal_sb[:, 1, :])

                def xin_src(tt):
                    if tt == 0:
                        return hal_sb[:, 0, :]
                    if tt == NT_EXT - 1:
                        return hal_sb[:, 1, :]
                    return x_sb[:, tt - 1, :]

                # LN + transpose -> hT [128, ndc, 768] feature-major
                hT = work.tile([128, NDC, EXT], F32R, tag="bigA")
                h_cache = {}

                def get_h(tt):
                    if tt not in h_cache:
                        ht = htok.tile([128, D], F32, tag="htok")
                        layernorm_tile(xin_src(tt), ht[:])
                        h_cache[tt] = ht
                    return h_cache[tt]

                transpose_block(get_h, NT_EXT, hT)
                if s_outer == 0 and s_inner == 0:
                    nc.gpsimd.dma_start(out=dbg_d[4], in_=hT[:, 0, 0:D])

                # qT, kT feature-major; v token-major bf16
                qT = work.tile([128, NDC, C], F32R, tag="qT")
                kT = work.tile([128, NDC, EXT], F32R, tag="bigB")
                for ot in range(NDC):
                    ps = psA.tile([128, C], F32, tag="a")
                    for dc in range(NDC):
                        nc.tensor.matmul(
                            ps[:], wq_sb[:, dc, ot * 128:(ot + 1) * 128],
                            hT[:, dc, 128:128 + C],
                            start=(dc == 0), stop=(dc == NDC - 1))
                    nc.vector.tensor_scalar_add(out=qT[:, ot, :], in0=ps[:],
                                                scalar1=bqk_sb[:, ot, 0:1])
                    for (a0, a1) in [(0, 512), (512, 768)]:
                        ps2 = psA.tile([128, 512], F32, tag="a")
                        for dc in range(NDC):
                            nc.tensor.matmul(
                                ps2[:, :a1 - a0],
                                wk_sb[:, dc, ot * 128:(ot + 1) * 128],
                                hT[:, dc, a0:a1],
                                start=(dc == 0), stop=(dc == NDC - 1))
                        nc.vector.tensor_scalar_add(
                            out=kT[:, ot, a0:a1], in0=ps2[:, :a1 - a0],
                            scalar1=bqk_sb[:, ot, 1:2])
                for tt in range(NT_EXT):
                    ps = psA.tile([128, D], F32, tag="a")
                    for dc in range(NDC):
                        nc.tensor.matmul(
                            ps[:], hT[:, dc, tt * 128:(tt + 1) * 128],
                            wv_sb[:, dc, :], start=(dc == 0),
                            stop=(dc == NDC - 1))
                    nc.vector.tensor_tensor(
                        out=v_sb[:, tt, :, 0:DH],
                        in0=ps[:].rearrange("p (h d) -> p h d", h=H),
                        in1=bv_sb[:].rearrange("p (h d) -> p h d", h=H),
                        op=ALU.add)

                if s_outer == 0 and s_inner == 0:
                    nc.gpsimd.dma_start(out=dbg_d[5], in_=qT[:, 0, :])
                    nc.gpsimd.dma_start(out=dbg_d[6], in_=kT[:, 0, 0:D])
                # banded attention in transposed layout
                oT = work.tile([64, H, C], BF16, tag="oT")
                for h in range(H):
                    hp = (h % 2) * 64
                    hc = h // 2
                    o_ps = psO.tile([DH, C], F32, tag="o")
                    d_ps = psO.tile([1, C], F32, tag="d")
                    for j in range(NT_EXT):
                        qlo, qhi = QWIN[j]
                        wj = qhi - qlo
                        s_ps = psS.tile([128, 384], F32, tag="s")
                        nc.tensor.matmul(
                            s_ps[:, :wj],
                            kT[hp:hp + 64, hc, j * 128:(j + 1) * 128],
                            qT[hp:hp + 64, hc, qlo:qhi], start=True, stop=True)
                        nc.scalar.activation(out=pT_sb[:, j, qlo:qhi],
                                             in_=s_ps[:, :wj],
                                             func=AF.Exp, scale=1.0 / 8.0)
                        nc.gpsimd.tensor_tensor(
                            out=pT_sb[:, j, :], in0=pT_sb[:, j, :],
                            in1=mask_sb[:, j, :], op=ALU.mult)
                    for j in range(NT_EXT):
                        nc.tensor.matmul(
                            o_ps[:], v_sb[:, j, h, 0:DH], pT_sb[:, j, :],
                            start=(j == 0), stop=(j == NT_EXT - 1))
                    for j in range(NT_EXT):
                        nc.tensor.matmul(
                            d_ps[:], v_sb[:, j, h, DH:DH + 1], pT_sb[:, j, :],
                            start=(j == 0), stop=(j == NT_EXT - 1))
                    rec = small.tile([1, C], F32R, tag="rec")
                    nc.vector.reciprocal(out=rec[:], in_=d_ps[0:1, :])
                    rb_ps = psS.tile([64, C], F32, tag="s")
                    nc.tensor.matmul(rb_ps[:], ones_sb[0:1, 0:64], rec[:],
                                     start=True, stop=True)
                    rb = htok.tile([64, C], F32, tag="rb")
                    nc.scalar.copy(out=rb[:], in_=rb_ps[:])
                    nc.vector.tensor_tensor(out=oT[:, h, :], in0=o_ps[:],
                                            in1=rb[:], op=ALU.mult)

                if s_outer == 0 and s_inner == 0:
                    dbgo = htok.tile([64, C], F32, tag="rb")
                    nc.scalar.copy(out=dbgo[:], in_=oT[:, 0, :])
                    nc.sync.dma_start(out=dbg_d[7, 0:64, :], in_=dbgo[:])
                # x += oT.T @ wo   (two heads packed per contraction via row tiles)
                for tt in range(NT_OWN):
                    ps = psA.tile([128, D], F32, tag="a")
                    for h in range(H):
                        nc.tensor.matmul(
                            ps[:], oT[:, h, tt * 128:(tt + 1) * 128],
                            wo_sb[:, h, :],
                            start=(h == 0), stop=(h == H - 1))
                    nc.vector.tensor_add(out=x_sb[:, tt, :],
                                         in0=x_sb[:, tt, :], in1=ps[:])

                # MLP
                h2T = work.tile([128, NDC, C], F32R, tag="bigB")
                h2_cache = {}

                def get_h2(tt):
                    if tt not in h2_cache:
                        ht = htok.tile([128, D], F32, tag="htok")
                        layernorm_tile(x_sb[:, tt, :], ht[:])
                        h2_cache[tt] = ht
                    return h2_cache[tt]

                transpose_block(get_h2, NT_OWN, h2T)
                gT = work.tile([128, 8, C], F32R, tag="bigA")
                for gt in range(8):
                    ps = psA.tile([128, C], F32, tag="a")
                    for dc in range(NDC):
                        nc.tensor.matmul(
                            ps[:], wg_sb[:, dc, gt * 128:(gt + 1) * 128],
                            h2T[:, dc, :], start=(dc == 0), stop=(dc == NDC - 1))
                    nc.scalar.activation(out=gT[:, gt, :], in_=ps[:],
                                         func=AF.Silu,
                                         bias=bg_sb[:, gt:gt + 1], scale=1.0)
                for tt in range(NT_OWN):
                    ps = psA.tile([128, D], F32, tag="a")
                    for gt in range(8):
                        nc.tensor.matmul(
                            ps[:], gT[:, gt, tt * 128:(tt + 1) * 128],
                            wd_sb[:, gt, :], start=(gt == 0), stop=(gt == 7))
                    nc.vector.tensor_add(out=x_sb[:, tt, :],
                                         in0=x_sb[:, tt, :], in1=ps[:])

            # ============ cross attention: z_H attends over z_L ============
            zlT = work.tile([128, NDC, C], F32R, tag="oTz")
            transpose_block(lambda tt: x_sb[:, tt, :], NT_OWN, zlT)

            hkw_t = work.tile([128, NDC, D], F32R, tag="bigA")
            hvw_t = work.tile([128, NDC, D], F32R, tag="bigB")
            nc.sync.dma_start(out=hkw_t[:], in_=hk_d[:])
            nc.sync.dma_start(out=hvw_t[:], in_=hv_d[:])

            hkT = work.tile([128, NDC, C], F32R, tag="qT")
            for ot in range(NDC):
                ps = psA.tile([128, C], F32, tag="a")
                for dc in range(NDC):
                    nc.tensor.matmul(
                        ps[:], hkw_t[:, dc, ot * 128:(ot + 1) * 128],
                        zlT[:, dc, :], start=(dc == 0), stop=(dc == NDC - 1))
                nc.scalar.copy(out=hkT[:, ot, :], in_=ps[:])
            for tt in range(NT_OWN):
                ps = psA.tile([128, D], F32, tag="a")
                for dc in range(NDC):
                    nc.tensor.matmul(
                        ps[:], zlT[:, dc, tt * 128:(tt + 1) * 128],
                        hvw_t[:, dc, :], start=(dc == 0), stop=(dc == NDC - 1))
                nc.scalar.copy(
                    out=v2_sb[:, tt, :, 0:DH_X],
                    in_=ps[:].rearrange("p (h d) -> p h d", h=NH_X))

            hqw_t = work.tile([128, NDC, D], F32R, tag="oT")
            nc.sync.dma_start(out=hqw_t[:], in_=hq_d[:])
            zh_ln = htok.tile([NM, D], F32, tag="zhln")
            layernorm_tile(zh_sb[:], zh_ln[:], np_=NM)
            zhT_ps = psA.tile([128, NDC * NM], F32, tag="a")
            for dc in range(NDC):
                nc.tensor.transpose(
                    zhT_ps[:, dc * NM:(dc + 1) * NM],
                    zh_ln[:, dc * 128:(dc + 1) * 128], id_sb[0:NM, 0:NM])
            zhT = small.tile([128, NDC, NM], F32R, tag="zhT")
            nc.scalar.copy(out=zhT[:].rearrange("p a b -> p (a b)"),
                           in_=zhT_ps[:])
            hqT = small.tile([128, NH_X, NM], F32R, tag="hqT")
            for xh in range(NH_X):
                ps = psS.tile([128, NM], F32, tag="s")
                for dc in range(NDC):
                    nc.tensor.matmul(
                        ps[:], hqw_t[:, dc, xh * 128:(xh + 1) * 128],
                        zhT[:, dc, :], start=(dc == 0), stop=(dc == NDC - 1))
                nc.vector.tensor_scalar_add(out=hqT[:, xh, :], in0=ps[:],
                                            scalar1=bhq_sb[:, xh:xh + 1])

            for xh in range(NH_X):
                o_ps = psO.tile([DH_X, NM], F32, tag="o")
                d_ps = psO.tile([1, NM], F32, tag="d")
                for kt in range(NT_OWN):
                    s_ps = psS.tile([128, NM], F32, tag="s")
                    nc.tensor.matmul(
                        s_ps[:], hkT[:, xh, kt * 128:(kt + 1) * 128],
                        hqT[:, xh, :], start=True, stop=True)
                    px = small.tile([128, NM], BF16, tag="px")
                    nc.scalar.activation(out=px[:], in_=s_ps[:], func=AF.Exp,
                                         scale=float(1.0 / np.sqrt(DH_X)))
                    nc.tensor.matmul(o_ps[:], v2_sb[:, kt, xh, 0:DH_X], px[:],
                                     start=(kt == 0), stop=(kt == NT_OWN - 1))
                    nc.tensor.matmul(d_ps[:], v2_sb[:, kt, xh, DH_X:DH_X + 1],
                                     px[:], start=(kt == 0),
                                     stop=(kt == NT_OWN - 1))
                oxs = small.tile([DH_X, NM], F32, tag="oxs")
                nc.scalar.copy(out=oxs[:], in_=o_ps[:])
                dxs = small.tile([1, NM], F32, tag="dxs")
                nc.scalar.copy(out=dxs[:], in_=d_ps[:])
                nc.sync.dma_start(
                    out=ar_in[xh * DH_X:(xh + 1) * DH_X, :], in_=oxs[:])
                nc.sync.dma_start(
                    out=ar_in[NH_X * DH_X + xh:NH_X * DH_X + xh + 1, :],
                    in_=dxs[:])
            nc.gpsimd.collective_compute(
                "AllReduce", ALU.add, ins=[ar_in[:]], outs=[ar_out[:]],
                replica_groups=GROUPS)

            how_t = work.tile([128, NDC, D], F32R, tag="bigA")
            nc.sync.dma_start(out=how_t[:], in_=how_d[:])
            oxn = small.tile([128, NH_X, NM], F32R, tag="oxn")
            den = small.tile([1, NH_X * NM], F32R, tag="den")
            for xh in range(NH_X):
                nc.gpsimd.dma_start(
                    out=den[0:1, xh * NM:(xh + 1) * NM],
                    in_=ar_out[NH_X * DH_X + xh:NH_X * DH_X + xh + 1, :])
            nc.vector.reciprocal(out=den[:], in_=den[:])
            for xh in range(NH_X):
                ox = small.tile([128, NM], F32, tag="ox")
                nc.sync.dma_start(
                    out=ox[:], in_=ar_out[xh * DH_X:(xh + 1) * DH_X, :])
                rb_ps = psS.tile([128, NM], F32, tag="s")
                nc.tensor.matmul(rb_ps[:], ones_sb[0:1, :],
                                 den[0:1, xh * NM:(xh + 1) * NM],
                                 start=True, stop=True)
                rb = small.tile([128, NM], F32, tag="rb2")
                nc.scalar.copy(out=rb[:], in_=rb_ps[:])
                nc.vector.tensor_tensor(out=oxn[:, xh, :], in0=ox[:],
                                        in1=rb[:], op=ALU.mult)
            ho_ps = psO.tile([NM, D], F32, tag="o")
            for xh in range(NH_X):
                nc.tensor.matmul(ho_ps[:], oxn[:, xh, :], how_t[:, xh, :],
                                 start=(xh == 0), stop=(xh == NH_X - 1))
            nc.vector.tensor_add(out=zh_sb[:], in0=zh_sb[:], in1=ho_ps[:])

        # ================= output: LN(z_H @ proj_out) * g + b =================
        zhT2_ps = psA.tile([128, NDC * NM], F32, tag="a")
        for dc in range(NDC):
            nc.tensor.transpose(zhT2_ps[:, dc * NM:(dc + 1) * NM],
                                zh_sb[:, dc * 128:(dc + 1) * 128],
                                id_sb[0:NM, 0:NM])
        zhT2 = small.tile([128, NDC, NM], F32R, tag="zhT")
        nc.scalar.copy(out=zhT2[:].rearrange("p a b -> p (a b)"),
                       in_=zhT2_ps[:])
        sts = small.tile([NM, 8, 6], F32, tag="ysts")
        for ns in range(8):
            ps = psA.tile([NM, D], F32, tag="a")
            for dc in range(NDC):
                po_t = stream.tile([128, D], F32R, tag="pt")
                nc.sync.dma_start(out=po_t[:], in_=po_d[dc, ns])
                nc.tensor.matmul(ps[:], zhT2[:, dc, :], po_t[:],
                                 start=(dc == 0), stop=(dc == NDC - 1))
            ych = htok.tile([NM, D], F32, tag="ych")
            nc.scalar.copy(out=ych[:], in_=ps[:])
            nc.vector.bn_stats(out=sts[:, ns, :], in_=ych[:])
            nc.sync.dma_start(out=y_scr[:, ns * D:(ns + 1) * D], in_=ych[:])
        mv = small.tile([NM, 2], F32, tag="ymv")
        nc.vector.bn_aggr(out=mv[:], in_=sts[:])
        rs = small.tile([NM, 1], F32, tag="yrs")
        nc.scalar.activation(out=rs[:], in_=mv[:, 1:2], func=AF.Sqrt,
                             bias=eps_sb[0:NM, :], scale=1.0)
        nc.vector.reciprocal(out=rs[:], in_=rs[:])
        for ns in range(8):
            ych = htok.tile([NM, D], F32, tag="ych")
            nc.sync.dma_start(out=ych[:], in_=y_scr[:, ns * D:(ns + 1) * D])
            nc.vector.tensor_scalar(out=ych[:], in0=ych[:],
                                    scalar1=mv[:, 0:1], scalar2=rs[:],
                                    op0=ALU.subtract, op1=ALU.mult)
            ot = small.tile([NM, D], BF16, tag="ongc")
            nc.sync.dma_start(out=ot[:], in_=ong_d[:, ns * D:(ns + 1) * D])
            nc.vector.tensor_tensor(out=ych[:], in0=ych[:], in1=ot[:],
                                    op=ALU.mult)
            ob = small.tile([NM, D], BF16, tag="onbc")
            nc.sync.dma_start(out=ob[:], in_=onb_d[:, ns * D:(ns + 1) * D])
            nc.vector.tensor_tensor(out=ych[:], in0=ych[:], in1=ob[:],
                                    op=ALU.add)
            nc.sync.dma_start(out=out_y[:, ns * D:(ns + 1) * D], in_=ych[:])

    nc.compile()
    return nc


def _prep_inputs(inputs):
    f = lambda k: np.asarray(inputs[k], dtype=np.float32)
    prompt = f("prompt_embeddings")
    proj_in_w = f("proj_in_w")
    bn_g, bn_b = f("bn_g"), f("bn_b")
    wq, wk, wv, wo = f("wq"), f("wk"), f("wv"), f("wo")
    fn_g, fn_b = f("fn_g"), f("fn_b")
    wg, wd = f("wg"), f("wd")
    h_init = f("h_init")
    hn_g, hn_b = f("hn_g"), f("hn_b")
    hq_w, hk_w, hv_w, ho_w = f("hq_w"), f("hk_w"), f("hv_w"), f("ho_w")
    proj_out_w = f("proj_out_w")
    on_g, on_b = f("on_g"), f("on_b")

    def chunk_w(w):  # [K, N] -> [128, K//128, N]
        dk, n = w.shape
        return np.ascontiguousarray(
            w.reshape(dk // 128, 128, n).transpose(1, 0, 2))

    bq, bk, bv = bn_b @ wq, bn_b @ wk, bn_b @ wv
    bg = fn_b @ wg
    bhq = hn_b @ hq_w
    shared = {
        "proj_in": np.ascontiguousarray(proj_in_w.reshape(N_DIN, 128, D)),
        "wq": chunk_w(bn_g[:, None] * wq), "wk": chunk_w(bn_g[:, None] * wk),
        "wv": chunk_w(bn_g[:, None] * wv),
        "wo": np.ascontiguousarray(wo.reshape(H, 64, D).transpose(1, 0, 2)).astype(NPBF16),
        "wg": chunk_w(fn_g[:, None] * wg), "wd": chunk_w(wd),
        "hk_w": chunk_w(hk_w), "hv_w": chunk_w(hv_w),
        "hq_w": chunk_w(hn_g[:, None] * hq_w), "ho_w": chunk_w(ho_w),
        "proj_out": np.ascontiguousarray(
            proj_out_w.reshape(NDC, 128, 8, D).transpose(0, 2, 1, 3)),
        "bqk": np.ascontiguousarray(
            np.stack([bq.reshape(NDC, 128).T, bk.reshape(NDC, 128).T],
                     axis=-1)),
        "bg_t": np.ascontiguousarray(bg.reshape(8, 128).T),
        "bhq_t": np.ascontiguousarray(bhq.reshape(NDC, 128).T),
        "bv_bc": np.ascontiguousarray(np.tile(bv[None, :], (128, 1))),
        "identity": np.eye(128, dtype=np.float32),
        "onesc": np.ones((128, 128), np.float32),
        "on_g_bc": np.ascontiguousarray(
            np.tile(on_g[None, :], (NM, 1)).astype(NPBF16)),
        "on_b_bc": np.ascontiguousarray(
            np.tile(on_b[None, :], (NM, 1)).astype(NPBF16)),
    }

    in_maps = []
    r = np.arange(128)
    for core in range(8):
        b, c = core // NCHUNK, core % NCHUNK
        start = c * C
        m = dict(shared)
        m["promptT"] = np.ascontiguousarray(
            prompt[b, start:start + C, :].T).reshape(N_DIN, 128, C)
        masks = np.zeros((128, 6, C), np.float32)
        for j in range(6):
            kglob = start - W_WIN + j * 128 + r
            qglob = start + np.arange(C)
            valid = (kglob >= 0) & (kglob < T)
            band = np.abs(kglob[:, None] - qglob[None, :]) <= W_WIN
            masks[:, j, :] = band & valid[:, None]
        m["masks"] = masks.astype(NPBF16)
        li = ((c - 1) % NCHUNK) * 256 + 128 + r
        ri = ((c + 1) % NCHUNK) * 256 + r
        m["halo_idx"] = np.ascontiguousarray(
            np.stack([li, ri], axis=-1).astype(np.int32))
        m["h_init_bc"] = np.ascontiguousarray(
            np.broadcast_to(h_init[0], (NM, D)).astype(np.float32))
        in_maps.append(m)
    return in_maps


def kernel(**inputs):
    if "nc" not in _CACHE:
        _CACHE["nc"] = _build_program()
    nc = _CACHE["nc"]
    in_maps = _prep_inputs(inputs)
    trace = bool(os.environ.get("KBENCH_TRACE"))
    res = run_bass_kernel_spmd(nc, in_maps, core_ids=list(range(8)),
                               trace=trace)
    if trace and res.exec_time_ns is not None:
        print(f"HW exec time: {res.exec_time_ns} ns")
        _CACHE["exec_time_ns"] = res.exec_time_ns
    out = np.stack([res.results[0]["out_y"], res.results[4]["out_y"]], axis=0)
    return out.astype(np.float32)



# revision 1
# speedup vs baseline: 22.9008x; 22.9008x over previous
"""Trainium2 Bass kernel for nn_LocalSolverCore (sparse local-window attention solver).

Sharding: 8 cores = 2 batches x 4 sequence-chunks of 512 tokens.
Per transformer block: AllGather halo exchange (128 tokens each side) within
each batch group of 4 cores; banded attention computed in transposed score
layout [k_part, q_free]; probs/masks/V in bf16; matmuls in float32r.
LN gains/biases are folded into the following weight matrices host-side.
"""

import os
import numpy as np
import ml_dtypes

import concourse.bass as bass
import concourse.mybir as mybir
import concourse.tile as tile
from concourse import bacc
from concourse.bass_utils import run_bass_kernel_spmd

BF16 = mybir.dt.bfloat16
F32 = mybir.dt.float32
F32R = mybir.dt.float32r
I32 = mybir.dt.int32
NPBF16 = ml_dtypes.bfloat16
AF = mybir.ActivationFunctionType
ALU = mybir.AluOpType

B, T, D_IN, D = 2, 2048, 4096, 512
H, DH, W_WIN, NM = 8, 64, 128, 16
K_OUTER, K_INNER = 3, 4
NH_X, DH_X = 4, 128
EPS = 1e-5
C = 512
EXT = C + 2 * W_WIN          # 768
NT_OWN, NT_EXT = 4, 6
NDC = 4                      # D/128
N_DIN = 32                   # D_IN/128
NCHUNK = 4

# k-tile j (ext rows [128j,128j+128)) -> q window [qlo, qhi)
QWIN = [(0, 128), (0, 256), (0, 384), (128, 512), (256, 512), (384, 512)]
# PV first-writer splits per j: (qa, qb, start)
PV_SPLITS = [
    [(0, 128, True)],
    [(0, 128, False), (128, 256, True)],
    [(0, 256, False), (256, 384, True)],
    [(128, 384, False), (384, 512, True)],
    [(256, 512, False)],
    [(384, 512, False)],
]
GROUPS = [[0, 1, 2, 3], [4, 5, 6, 7]]

_CACHE = {}


def _build_program():
    nc = bacc.Bacc(None, target_bir_lowering=False)

    def inp(name, shape, dt=F32):
        return nc.dram_tensor(name, list(shape), dt, kind="ExternalInput")

    promptT_d = inp("promptT", [N_DIN, 128, C], F32R)
    proj_in_d = inp("proj_in", [N_DIN, 128, D], F32R)
    wq_d = inp("wq", [128, NDC, D], F32R)
    wk_d = inp("wk", [128, NDC, D], F32R)
    wv_d = inp("wv", [128, NDC, D], F32R)
    wo_d = inp("wo", [64, H, D], BF16)
    wg_d = inp("wg", [128, NDC, 2 * D], F32R)
    wd_d = inp("wd", [128, 8, D], F32R)
    hk_d = inp("hk_w", [128, NDC, D], F32R)
    hv_d = inp("hv_w", [128, NDC, D], F32R)
    hq_d = inp("hq_w", [128, NDC, D], F32R)
    how_d = inp("ho_w", [128, NDC, D], F32R)
    po_d = inp("proj_out", [NDC, 8, 128, D], F32R)
    bqk_d = inp("bqk", [128, NDC, 2])
    bg_d = inp("bg_t", [128, 8])
    bhq_d = inp("bhq_t", [128, NDC])
    bv_d = inp("bv_bc", [128, D])
    mask_d = inp("masks", [128, 6, C], BF16)
    hidx_d = inp("halo_idx", [128, 2], I32)
    id_d = inp("identity", [128, 128])
    ones_d = inp("onesc", [128, 128], F32R)
    zh0_d = inp("h_init_bc", [NM, D])
    ong_d = inp("on_g_bc", [NM, D_IN], BF16)
    onb_d = inp("on_b_bc", [NM, D_IN], BF16)

    out_y = nc.dram_tensor("out_y", [NM, D_IN], F32, kind="ExternalOutput")
    dbg_d = nc.dram_tensor("dbg", [8, 128, D], F32, kind="ExternalOutput")

    ag_in = nc.dram_tensor("ag_in", [256, D], F32)
    ag_out = nc.dram_tensor("ag_out", [1024, D], F32)
    ar_in = nc.dram_tensor("ar_in", [NH_X * DH_X + NH_X, NM], F32)
    ar_out = nc.dram_tensor("ar_out", [NH_X * DH_X + NH_X, NM], F32)
    y_scr = nc.dram_tensor("y_scr", [NM, D_IN], F32)

    import contextlib
    with nc.allow_low_precision(reason="bf16 probs/f32r matmul operands are intentional"), \
            tile.TileContext(nc) as tc, contextlib.ExitStack() as ctx:
        singles = ctx.enter_context(tc.tile_pool(name="singles", bufs=1))
        psA = ctx.enter_context(tc.tile_pool(name="psA", bufs=4, space="PSUM"))
        psS = ctx.enter_context(tc.tile_pool(name="psS", bufs=2, space="PSUM"))
        psO = ctx.enter_context(tc.tile_pool(name="psO", bufs=1, space="PSUM"))
        work = ctx.enter_context(tc.tile_pool(name="work", bufs=1))
        htok = ctx.enter_context(tc.tile_pool(name="htok", bufs=2))
        small = ctx.enter_context(tc.tile_pool(name="small", bufs=2))
        stream = ctx.enter_context(tc.tile_pool(name="stream", bufs=2))

        def load(name, ap, shape, dt=F32):
            t = singles.tile(list(shape), dt, tag=name)
            nc.sync.dma_start(out=t[:], in_=ap)
            return t

        wq_sb = load("wq", wq_d[:], [128, NDC, D], F32R)
        wk_sb = load("wk", wk_d[:], [128, NDC, D], F32R)
        wv_sb = load("wv", wv_d[:], [128, NDC, D], F32R)
        wo_sb = load("wo", wo_d[:], [64, H, D], BF16)
        wg_sb = load("wg", wg_d[:], [128, NDC, 2 * D], F32R)
        wd_sb = load("wd", wd_d[:], [128, 8, D], F32R)
        bqk_sb = load("bqk", bqk_d[:], [128, NDC, 2])
        bg_sb = load("bg_t", bg_d[:], [128, 8])
        bhq_sb = load("bhq_t", bhq_d[:], [128, NDC])
        bv_sb = load("bv_bc", bv_d[:], [128, D])
        mask_sb = load("masks", mask_d[:], [128, 6, C], BF16)
        hidx_sb = load("halo_idx", hidx_d[:], [128, 2], I32)
        id_sb = load("identity", id_d[:], [128, 128])
        ones_sb = load("onesc", ones_d[:], [128, 128], F32R)
        zh_sb = load("h_init_bc", zh0_d[:], [NM, D])

        eps_sb = singles.tile([128, 1], F32, tag="eps")
        nc.vector.memset(eps_sb[:], EPS)

        e_sb = singles.tile([128, NT_OWN, D], F32, tag="e")
        x_sb = singles.tile([128, NT_OWN, D], F32, tag="x")
        hal_sb = singles.tile([128, 2, D], F32, tag="hal")
        v_sb = singles.tile([128, NT_EXT, H, DH + 1], BF16, tag="v")
        v2_sb = singles.tile([128, NT_OWN, NH_X, DH_X + 1], BF16, tag="v2")
        nc.vector.memset(v_sb[:, :, :, DH:DH + 1], 1.0)
        pT_sb = singles.tile([128, 6, C], BF16, tag="pTs")
        nc.gpsimd.memset(pT_sb[:], 0.0)
        nc.vector.memset(v2_sb[:, :, :, DH_X:DH_X + 1], 1.0)

        # ----- e = promptT.T @ proj_in (token-major); z_L = 0.1 e -----
        e_ps = [psA.tile([128, D], F32, tag="a", name=f"e_ps{i}") for i in range(NT_OWN)]
        for dc in range(N_DIN):
            pt_t = stream.tile([128, C], F32R, tag="pt")
            pi_t = stream.tile([128, D], F32R, tag="pi")
            nc.sync.dma_start(out=pt_t[:], in_=promptT_d[dc])
            nc.sync.dma_start(out=pi_t[:], in_=proj_in_d[dc])
            for tt in range(NT_OWN):
                nc.tensor.matmul(
                    e_ps[tt][:], pt_t[:, tt * 128:(tt + 1) * 128], pi_t[:],
                    start=(dc == 0), stop=(dc == N_DIN - 1))
        for tt in range(NT_OWN):
            nc.scalar.copy(out=e_sb[:, tt, :], in_=e_ps[tt][:])
            nc.scalar.mul(out=x_sb[:, tt, :], in_=e_ps[tt][:], mul=0.1)

        def layernorm_tile(src_ap, dst_ap, np_=128):
            st = small.tile([np_, 6], F32, tag="bnst")
            mv = small.tile([np_, 2], F32, tag="bnmv")
            rs = small.tile([np_, 1], F32, tag="rstd")
            nc.vector.bn_stats(out=st[:], in_=src_ap)
            nc.vector.bn_aggr(out=mv[:], in_=st[:])
            nc.scalar.activation(out=rs[:], in_=mv[:, 1:2], func=AF.Sqrt,
                                 bias=eps_sb[0:np_, :], scale=1.0)
            nc.vector.reciprocal(out=rs[:], in_=rs[:])
            nc.vector.tensor_scalar(out=dst_ap, in0=src_ap,
                                    scalar1=mv[:, 0:1], scalar2=rs[:],
                                    op0=ALU.subtract, op1=ALU.mult)

        def transpose_block(get_src, n_tt, dst_sb):
            """dst_sb[:, ds, tt*128+...] = src(tt)[:, ds*128+...].T via PE."""
            for t0 in range(0, n_tt, 4):
                tts = list(range(t0, min(t0 + 4, n_tt)))
                pss = [psA.tile([128, 512], F32, tag="a", name=f"tp{i}") for i in range(NDC)]
                for tt in tts:
                    src = get_src(tt)
                    for ds in range(NDC):
                        nc.tensor.transpose(
                            pss[ds][:, (tt - t0) * 128:(tt - t0 + 1) * 128],
                            src[:, ds * 128:(ds + 1) * 128], id_sb[:])
                w = len(tts) * 128
                for ds in range(NDC):
                    nc.scalar.copy(out=dst_sb[:, ds, t0 * 128:t0 * 128 + w],
                                   in_=pss[ds][:, :w])

        # ================= main iteration =================
        for s_outer in range(K_OUTER):
            for s_inner in range(K_INNER):
                # x_in = z_L + e  (in place; the +e is part of the block output)
                for tt in range(NT_OWN):
                    nc.vector.tensor_add(out=x_sb[:, tt, :],
                                         in0=x_sb[:, tt, :], in1=e_sb[:, tt, :])
                # halo exchange of boundary token tiles
                nc.sync.dma_start(out=ag_in[0:128, :], in_=x_sb[:, 0, :])
                nc.sync.dma_start(out=ag_in[128:256, :], in_=x_sb[:, 3, :])
                nc.gpsimd.collective_compute(
                    "AllGather", ALU.bypass, ins=[ag_in[:]], outs=[ag_out[:]],
                    replica_groups=GROUPS)
                nc.gpsimd.indirect_dma_start(
                    out=hal_sb[:, 0, :], out_offset=None, in_=ag_out[:],
                    in_offset=bass.IndirectOffsetOnAxis(ap=hidx_sb[:, 0:1],
                                                        axis=0))
                nc.gpsimd.indirect_dma_start(
                    out=hal_sb[:, 1, :], out_offset=None, in_=ag_out[:],
                    in_offset=bass.IndirectOffsetOnAxis(ap=hidx_sb[:, 1:2],
                                                        axis=0))

                if s_outer == 0 and s_inner == 0:
                    nc.sync.dma_start(out=dbg_d[0], in_=e_sb[:, 0, :])
                    nc.sync.dma_start(out=dbg_d[1], in_=x_sb[:, 0, :])
                    nc.sync.dma_start(out=dbg_d[2], in_=hal_sb[:, 0, :])
                    nc.sync.dma_start(out=dbg_d[3], in_=hal_sb[:, 1, :])

                def xin_src(tt):
                    if tt == 0:
                        return hal_sb[:, 0, :]
                    if tt == NT_EXT - 1:
                        return hal_sb[:, 1, :]
                    return x_sb[:, tt - 1, :]

                # LN + transpose -> hT [128, ndc, 768] feature-major
                hT = work.tile([128, NDC, EXT], F32R, tag="bigA")
                h_cache = {}

                def get_h(tt):
                    if tt not in h_cache:
                        ht = htok.tile([128, D], F32, tag="htok")
                        layernorm_tile(xin_src(tt), ht[:])
                        h_cache[tt] = ht
                    return h_cache[tt]

                transpose_block(get_h, NT_EXT, hT)
                if s_outer == 0 and s_inner == 0:
                    nc.gpsimd.dma_start(out=dbg_d[4], in_=hT[:, 0, 0:D])

                # qT, kT feature-major; v token-major bf16
                qT = work.tile([128, NDC, C], F32R, tag="qT")
                kT = work.tile([128, NDC, EXT], F32R, tag="bigB")
                for ot in range(NDC):
                    ps = psA.tile([128, C], F32, tag="a")
                    for dc in range(NDC):
                        nc.tensor.matmul(
                            ps[:], wq_sb[:, dc, ot * 128:(ot + 1) * 128],
                            hT[:, dc, 128:128 + C],
                            start=(dc == 0), stop=(dc == NDC - 1))
                    nc.vector.tensor_scalar_add(out=qT[:, ot, :], in0=ps[:],
                                                scalar1=bqk_sb[:, ot, 0:1])
                    for (a0, a1) in [(0, 512), (512, 768)]:
                        ps2 = psA.tile([128, 512], F32, tag="a")
                        for dc in range(NDC):
                            nc.tensor.matmul(
                                ps2[:, :a1 - a0],
                                wk_sb[:, dc, ot * 128:(ot + 1) * 128],
                                hT[:, dc, a0:a1],
                                start=(dc == 0), stop=(dc == NDC - 1))
                        nc.vector.tensor_scalar_add(
                            out=kT[:, ot, a0:a1], in0=ps2[:, :a1 - a0],
                            scalar1=bqk_sb[:, ot, 1:2])
                for tt in range(NT_EXT):
                    ps = psA.tile([128, D], F32, tag="a")
                    for dc in range(NDC):
                        nc.tensor.matmul(
                            ps[:], hT[:, dc, tt * 128:(tt + 1) * 128],
                            wv_sb[:, dc, :], start=(dc == 0),
                            stop=(dc == NDC - 1))
                    nc.vector.tensor_tensor(
                        out=v_sb[:, tt, :, 0:DH],
                        in0=ps[:].rearrange("p (h d) -> p h d", h=H),
                        in1=bv_sb[:].rearrange("p (h d) -> p h d", h=H),
                        op=ALU.add)

                if s_outer == 0 and s_inner == 0:
                    nc.gpsimd.dma_start(out=dbg_d[5], in_=qT[:, 0, :])
                    nc.gpsimd.dma_start(out=dbg_d[6], in_=kT[:, 0, 0:D])
                # banded attention in transposed layout
                oT = work.tile([64, H, C], BF16, tag="oT")
                for h in range(H):
                    hp = (h % 2) * 64
                    hc = h // 2
                    o_ps = psO.tile([DH, C], F32, tag="o")
                    d_ps = psO.tile([1, C], F32, tag="d")
                    for j in range(NT_EXT):
                        qlo, qhi = QWIN[j]
                        wj = qhi - qlo
                        s_ps = psS.tile([128, 384], F32, tag="s")
                        nc.tensor.matmul(
                            s_ps[:, :wj],
                            kT[hp:hp + 64, hc, j * 128:(j + 1) * 128],
                            qT[hp:hp + 64, hc, qlo:qhi], start=True, stop=True)
                        nc.scalar.activation(out=pT_sb[:, j, qlo:qhi],
                                             in_=s_ps[:, :wj],
                                             func=AF.Exp, scale=1.0 / 8.0)
                        nc.gpsimd.tensor_tensor(
                            out=pT_sb[:, j, :], in0=pT_sb[:, j, :],
                            in1=mask_sb[:, j, :], op=ALU.mult)
                    for j in range(NT_EXT):
                        nc.tensor.matmul(
                            o_ps[:], v_sb[:, j, h, 0:DH], pT_sb[:, j, :],
                            start=(j == 0), stop=(j == NT_EXT - 1))
                    for j in range(NT_EXT):
                        nc.tensor.matmul(
                            d_ps[:], v_sb[:, j, h, DH:DH + 1], pT_sb[:, j, :],
                            start=(j == 0), stop=(j == NT_EXT - 1))
                    rec = small.tile([1, C], F32R, tag="rec")
                    nc.vector.reciprocal(out=rec[:], in_=d_ps[0:1, :])
                    rb_ps = psS.tile([64, C], F32, tag="s")
                    nc.tensor.matmul(rb_ps[:], ones_sb[0:1, 0:64], rec[:],
                                     start=True, stop=True)
                    rb = htok.tile([64, C], F32, tag="rb")
                    nc.scalar.copy(out=rb[:], in_=rb_ps[:])
                    nc.vector.tensor_tensor(out=oT[:, h, :], in0=o_ps[:],
                                            in1=rb[:], op=ALU.mult)

                if s_outer == 0 and s_inner == 0:
                    dbgo = htok.tile([64, C], F32, tag="rb")
                    nc.scalar.copy(out=dbgo[:], in_=oT[:, 0, :])
                    nc.sync.dma_start(out=dbg_d[7, 0:64, :], in_=dbgo[:])
                # x += oT.T @ wo   (two heads packed per contraction via row tiles)
                for tt in range(NT_OWN):
                    ps = psA.tile([128, D], F32, tag="a")
                    for h in range(H):
                        nc.tensor.matmul(
                            ps[:], oT[:, h, tt * 128:(tt + 1) * 128],
                            wo_sb[:, h, :],
                            start=(h == 0), stop=(h == H - 1))
                    nc.vector.tensor_add(out=x_sb[:, tt, :],
                                         in0=x_sb[:, tt, :], in1=ps[:])

                # MLP
                h2T = work.tile([128, NDC, C], F32R, tag="bigB")
                h2_cache = {}

                def get_h2(tt):
                    if tt not in h2_cache:
                        ht = htok.tile([128, D], F32, tag="htok")
                        layernorm_tile(x_sb[:, tt, :], ht[:])
                        h2_cache[tt] = ht
                    return h2_cache[tt]

                transpose_block(get_h2, NT_OWN, h2T)
                gT = work.tile([128, 8, C], F32R, tag="bigA")
                for gt in range(8):
                    ps = psA.tile([128, C], F32, tag="a")
                    for dc in range(NDC):
                        nc.tensor.matmul(
                            ps[:], wg_sb[:, dc, gt * 128:(gt + 1) * 128],
                            h2T[:, dc, :], start=(dc == 0), stop=(dc == NDC - 1))
                    nc.scalar.activation(out=gT[:, gt, :], in_=ps[:],
                                         func=AF.Silu,
                                         bias=bg_sb[:, gt:gt + 1], scale=1.0)
                for tt in range(NT_OWN):
                    ps = psA.tile([128, D], F32, tag="a")
                    for gt in range(8):
                        nc.tensor.matmul(
                            ps[:], gT[:, gt, tt * 128:(tt + 1) * 128],
                            wd_sb[:, gt, :], start=(gt == 0), stop=(gt == 7))
                    nc.vector.tensor_add(out=x_sb[:, tt, :],
                                         in0=x_sb[:, tt, :], in1=ps[:])

            # ============ cross attention: z_H attends over z_L ============
            zlT = work.tile([128, NDC, C], F32R, tag="oTz")
            transpose_block(lambda tt: x_sb[:, tt, :], NT_OWN, zlT)

            hkw_t = work.tile([128, NDC, D], F32R, tag="bigA")
            hvw_t = work.tile([128, NDC, D], F32R, tag="bigB")
            nc.sync.dma_start(out=hkw_t[:], in_=hk_d[:])
            nc.sync.dma_start(out=hvw_t[:], in_=hv_d[:])

            hkT = work.tile([128, NDC, C], F32R, tag="qT")
            for ot in range(NDC):
                ps = psA.tile([128, C], F32, tag="a")
                for dc in range(NDC):
                    nc.tensor.matmul(
                        ps[:], hkw_t[:, dc, ot * 128:(ot + 1) * 128],
                        zlT[:, dc, :], start=(dc == 0), stop=(dc == NDC - 1))
                nc.scalar.copy(out=hkT[:, ot, :], in_=ps[:])
            for tt in range(NT_OWN):
                ps = psA.tile([128, D], F32, tag="a")
                for dc in range(NDC):
                    nc.tensor.matmul(
                        ps[:], zlT[:, dc, tt * 128:(tt + 1) * 128],
                        hvw_t[:, dc, :], start=(dc == 0), stop=(dc == NDC - 1))
                nc.scalar.copy(
                    out=v2_sb[:, tt, :, 0:DH_X],
                    in_=ps[:].rearrange("p (h d) -> p h d", h=NH_X))

            hqw_t = work.tile([128, NDC, D], F32R, tag="oT")
            nc.sync.dma_start(out=hqw_t[:], in_=hq_d[:])
            zh_ln = htok.tile([NM, D], F32, tag="zhln")
            layernorm_tile(zh_sb[:], zh_ln[:], np_=NM)
            zhT_ps = psA.tile([128, NDC * NM], F32, tag="a")
            for dc in range(NDC):
                nc.tensor.transpose(
                    zhT_ps[:, dc * NM:(dc + 1) * NM],
                    zh_ln[:, dc * 128:(dc + 1) * 128], id_sb[0:NM, 0:NM])
            zhT = small.tile([128, NDC, NM], F32R, tag="zhT")
            nc.scalar.copy(out=zhT[:].rearrange("p a b -> p (a b)"),
                           in_=zhT_ps[:])
            hqT = small.tile([128, NH_X, NM], F32R, tag="hqT")
            for xh in range(NH_X):
                ps = psS.tile([128, NM], F32, tag="s")
                for dc in range(NDC):
                    nc.tensor.matmul(
                        ps[:], hqw_t[:, dc, xh * 128:(xh + 1) * 128],
                        zhT[:, dc, :], start=(dc == 0), stop=(dc == NDC - 1))
                nc.vector.tensor_scalar_add(out=hqT[:, xh, :], in0=ps[:],
                                            scalar1=bhq_sb[:, xh:xh + 1])

            for xh in range(NH_X):
                o_ps = psO.tile([DH_X, NM], F32, tag="o")
                d_ps = psO.tile([1, NM], F32, tag="d")
                for kt in range(NT_OWN):
                    s_ps = psS.tile([128, NM], F32, tag="s")
                    nc.tensor.matmul(
                        s_ps[:], hkT[:, xh, kt * 128:(kt + 1) * 128],
                        hqT[:, xh, :], start=True, stop=True)
                    px = small.tile([128, NM], BF16, tag="px")
                    nc.scalar.activation(out=px[:], in_=s_ps[:], func=AF.Exp,
                                         scale=float(1.0 / np.sqrt(DH_X)))
                    nc.tensor.matmul(o_ps[:], v2_sb[:, kt, xh, 0:DH_X], px[:],
                                     start=(kt == 0), stop=(kt == NT_OWN - 1))
                    nc.tensor.matmul(d_ps[:], v2_sb[:, kt, xh, DH_X:DH_X + 1],
                                     px[:], start=(kt == 0),
                                     stop=(kt == NT_OWN - 1))
                oxs = small.tile([DH_X, NM], F32, tag="oxs")
                nc.scalar.copy(out=oxs[:], in_=o_ps[:])
                dxs = small.tile([1, NM], F32, tag="dxs")
                nc.scalar.copy(out=dxs[:], in_=d_ps[:])
                nc.sync.dma_start(
                    out=ar_in[xh * DH_X:(xh + 1) * DH_X, :], in_=oxs[:])
                nc.sync.dma_start(
                    out=ar_in[NH_X * DH_X + xh:NH_X * DH_X + xh + 1, :],
                    in_=dxs[:])
            nc.gpsimd.collective_compute(
                "AllReduce", ALU.add, ins=[ar_in[:]], outs=[ar_out[:]],
                replica_groups=GROUPS)

            how_t = work.tile([128, NDC, D], F32R, tag="bigA")
            nc.sync.dma_start(out=how_t[:], in_=how_d[:])
            oxn = small.tile([128, NH_X, NM], F32R, tag="oxn")
            den = small.tile([1, NH_X * NM], F32R, tag="den")
            for xh in range(NH_X):
                nc.gpsimd.dma_start(
                    out=den[0:1, xh * NM:(xh + 1) * NM],
                    in_=ar_out[NH_X * DH_X + xh:NH_X * DH_X + xh + 1, :])
            nc.vector.reciprocal(out=den[:], in_=den[:])
            for xh in range(NH_X):
                ox = small.tile([128, NM], F32, tag="ox")
                nc.sync.dma_start(
                    out=ox[:], in_=ar_out[xh * DH_X:(xh + 1) * DH_X, :])
                rb_ps = psS.tile([128, NM], F32, tag="s")
                nc.tensor.matmul(rb_ps[:], ones_sb[0:1, :],
                                 den[0:1, xh * NM:(xh + 1) * NM],
                                 start=True, stop=True)
                rb = small.tile([128, NM], F32, tag="rb2")
                nc.scalar.copy(out=rb[:], in_=rb_ps[:])
                nc.vector.tensor_tensor(out=oxn[:, xh, :], in0=ox[:],
                                        in1=rb[:], op=ALU.mult)
            ho_ps = psO.tile([NM, D], F32, tag="o")
            for xh in range(NH_X):
                nc.tensor.matmul(ho_ps[:], oxn[:, xh, :], how_t[:, xh, :],
                                 start=(xh == 0), stop=(xh == NH_X - 1))
            nc.vector.tensor_add(out=zh_sb[:], in0=zh_sb[:], in1=ho_ps[:])

        # ================= output: LN(z_H @ proj_out) * g + b =================
        zhT2_ps = psA.tile([128, NDC * NM], F32, tag="a")
        for dc in range(NDC):
            nc.tensor.transpose(zhT2_ps[:, dc * NM:(dc + 1) * NM],
                                zh_sb[:, dc * 128:(dc + 1) * 128],
                                id_sb[0:NM, 0:NM])
        zhT2 = small.tile([128, NDC, NM], F32R, tag="zhT")
        nc.scalar.copy(out=zhT2[:].rearrange("p a b -> p (a b)"),
                       in_=zhT2_ps[:])
        sts = small.tile([NM, 8, 6], F32, tag="ysts")
        for ns in range(8):
            ps = psA.tile([NM, D], F32, tag="a")
            for dc in range(NDC):
                po_t = stream.tile([128, D], F32R, tag="pt")
                nc.sync.dma_start(out=po_t[:], in_=po_d[dc, ns])
                nc.tensor.matmul(ps[:], zhT2[:, dc, :], po_t[:],
                                 start=(dc == 0), stop=(dc == NDC - 1))
            ych = htok.tile([NM, D], F32, tag="ych")
            nc.scalar.copy(out=ych[:], in_=ps[:])
            nc.vector.bn_stats(out=sts[:, ns, :], in_=ych[:])
            nc.sync.dma_start(out=y_scr[:, ns * D:(ns + 1) * D], in_=ych[:])
        mv = small.tile([NM, 2], F32, tag="ymv")
        nc.vector.bn_aggr(out=mv[:], in_=sts[:])
        rs = small.tile([NM, 1], F32, tag="yrs")
        nc.scalar.activation(out=rs[:], in_=mv[:, 1:2], func=AF.Sqrt,
                             bias=eps_sb[0:NM, :], scale=1.0)
        nc.vector.reciprocal(out=rs[:], in_=rs[:])
        for ns in range(8):
            ych = htok.tile([NM, D], F32, tag="ych")
            nc.sync.dma_start(out=ych[:], in_=y_scr[:, ns * D:(ns + 1) * D])
            nc.vector.tensor_scalar(out=ych[:], in0=ych[:],
                                    scalar1=mv[:, 0:1], scalar2=rs[:],
                                    op0=ALU.subtract, op1=ALU.mult)
            ot = small.tile([NM, D], BF16, tag="ongc")
            nc.sync.dma_start(out=ot[:], in_=ong_d[:, ns * D:(ns + 1) * D])
            nc.vector.tensor_tensor(out=ych[:], in0=ych[:], in1=ot[:],
                                    op=ALU.mult)
            ob = small.tile([NM, D], BF16, tag="onbc")
            nc.sync.dma_start(out=ob[:], in_=onb_d[:, ns * D:(ns + 1) * D])
            nc.vector.tensor_tensor(out=ych[:], in0=ych[:], in1=ob[:],
                                    op=ALU.add)
            nc.sync.dma_start(out=out_y[:, ns * D:(ns + 1) * D], in_=ych[:])

    nc.compile()
    return nc


def _prep_inputs(inputs):
    f = lambda k: np.asarray(inputs[k], dtype=np.float32)
    prompt = f("prompt_embeddings")
    proj_in_w = f("proj_in_w")
    bn_g, bn_b = f("bn_g"), f("bn_b")
    wq, wk, wv, wo = f("wq"), f("wk"), f("wv"), f("wo")
    fn_g, fn_b = f("fn_g"), f("fn_b")
    wg, wd = f("wg"), f("wd")
    h_init = f("h_init")
    hn_g, hn_b = f("hn_g"), f("hn_b")
    hq_w, hk_w, hv_w, ho_w = f("hq_w"), f("hk_w"), f("hv_w"), f("ho_w")
    proj_out_w = f("proj_out_w")
    on_g, on_b = f("on_g"), f("on_b")

    def chunk_w(w):  # [K, N] -> [128, K//128, N]
        dk, n = w.shape
        return np.ascontiguousarray(
            w.reshape(dk // 128, 128, n).transpose(1, 0, 2))

    bq, bk, bv = bn_b @ wq, bn_b @ wk, bn_b @ wv
    bg = fn_b @ wg
    bhq = hn_b @ hq_w
    shared = {
        "proj_in": np.ascontiguousarray(proj_in_w.reshape(N_DIN, 128, D)),
        "wq": chunk_w(bn_g[:, None] * wq), "wk": chunk_w(bn_g[:, None] * wk),
        "wv": chunk_w(bn_g[:, None] * wv),
        "wo": np.ascontiguousarray(wo.reshape(H, 64, D).transpose(1, 0, 2)).astype(NPBF16),
        "wg": chunk_w(fn_g[:, None] * wg), "wd": chunk_w(wd),
        "hk_w": chunk_w(hk_w), "hv_w": chunk_w(hv_w),
        "hq_w": chunk_w(hn_g[:, None] * hq_w), "ho_w": chunk_w(ho_w),
        "proj_out": np.ascontiguousarray(
            proj_out_w.reshape(NDC, 128, 8, D).transpose(0, 2, 1, 3)),
        "bqk": np.ascontiguousarray(
            np.stack([bq.reshape(NDC, 128).T, bk.reshape(NDC, 128).T],
                     axis=-1)),
        "bg_t": np.ascontiguousarray(bg.reshape(8, 128).T),
        "bhq_t": np.ascontiguousarray(bhq.reshape(NDC, 128).T),
        "bv_bc": np.ascontiguousarray(np.tile(bv[None, :], (128, 1))),
        "identity": np.eye(128, dtype=np.float32),
        "onesc": np.ones((128, 128), np.float32),
        "on_g_bc": np.ascontiguousarray(
            np.tile(on_g[None, :], (NM, 1)).astype(NPBF16)),
        "on_b_bc": np.ascontiguousarray(
            np.tile(on_b[None, :], (NM, 1)).astype(NPBF16)),
    }

    in_maps = []
    r = np.arange(128)
    for core in range(8):
        b, c = core // NCHUNK, core % NCHUNK
        start = c * C
        m = dict(shared)
        m["promptT"] = np.ascontiguousarray(
            prompt[b, start:start + C, :].T).reshape(N_DIN, 128, C)
        masks = np.zeros((128, 6, C), np.float32)
        for j in range(6):
            kglob = start - W_WIN + j * 128 + r
            qglob = start + np.arange(C)
            valid = (kglob >= 0) & (kglob < T)
            band = np.abs(kglob[:, None] - qglob[None, :]) <= W_WIN
            masks[:, j, :] = band & valid[:, None]
        m["masks"] = masks.astype(NPBF16)
        li = ((c - 1) % NCHUNK) * 256 + 128 + r
        ri = ((c + 1) % NCHUNK) * 256 + r
        m["halo_idx"] = np.ascontiguousarray(
            np.stack([li, ri], axis=-1).astype(np.int32))
        m["h_init_bc"] = np.ascontiguousarray(
            np.broadcast_to(h_init[0], (NM, D)).astype(np.float32))
        in_maps.append(m)
    return in_maps


def kernel(**inputs):
    if "nc" not in _CACHE:
        _CACHE["nc"] = _build_program()
    nc = _CACHE["nc"]
    in_maps = _prep_inputs(inputs)
    trace = bool(os.environ.get("KBENCH_TRACE"))
    res = run_bass_kernel_spmd(nc, in_maps, core_ids=list(range(8)),
                               trace=trace)
    if trace and res.exec_time_ns is not None:
        print(f"HW exec time: {res.exec_time_ns} ns")
        _CACHE["exec_time_ns"] = res.exec_time_ns
    out = np.stack([res.results[0]["out_y"], res.results[4]["out_y"]], axis=0)
    return out.astype(np.float32)

